# revision 34
# baseline (speedup 1.0000x reference)
"""PoissonGaussianReadout forward on 8 trn2 NeuronCores.

Math (eval mode): each neuron n samples feat[b] (a [36,36,1024] image per
batch, 1024 = C*T channels) bilinearly at a fixed point mu[n], then takes a
per-neuron dot with W[n,:], adds b[n], applies elu(y)+1.

Strategy:
  - Hybrid shard 4x2: 8 cores = 4 batch-groups (4 batches each) x 2 halves
    of the contraction dim D (512 channels each).  Cores emit LINEAR
    partial sums; the host adds the halves, bias, and elu on [16,4096].
    (Splitting D halves the per-core W traffic; splitting batches keeps
    the per-core feat traffic at 1/8 -- together they minimize both the
    DMA stream and the feat-arrival gate.)
  - fp8(e4m3) x and W with DoubleRow matmuls: x uses one global scale, W a
    per-neuron scale; both dequant factors fold into the (per-neuron) mask.
    Halves both the DMA stream and the PE time vs bf16 (rel err 1.3e-2,
    within the 2e-2 gate; inputs are deterministic).
  - Sort neurons by bilinear base cell p00 = y0*36+x0; blocks of <=128
    sorted neurons span a window of <=WINMAX flat positions.  Two
    DoubleRow matmuls per block (256-channel subtile pairs):
    psum[n, (b,j)] += Wblk^T @ feat-window, a contiguous slice of feat.
  - Each neuron's 4 bilinear corners live at window offsets
    (p00-pfirst)+{0,1,36,37}; a host-built sparse mask [n, win] (bf16,
    fp8 dequant folded in) holds the bilinear weights.  One DVE
    scalar_tensor_tensor per (block, batch) fuses mask-multiply and
    window-reduce straight out of PSUM into z.  This DVE phase (~21us)
    is the critical path: it is the only engine that can do
    tensor*tensor reads from PSUM, and its cost is bound by
    per-instruction window elements + accumulator drains.
  - DMA is need-ordered on the two HWDGE queues: one contiguous feat
    double-chunk per queue first (the whole PE/DVE pipeline gates on
    feat), then W block-groups + their mask slices just-in-time behind
    PE consumption; per-group z stores overlap the pipeline.
"""
import sys
sys.path.insert(0, "/opt/trn_rl_repo")

import numpy as np

from concourse import bass, mybir, tile
from concourse.bass_utils import run_bass_kernel_spmd
import bass_rust

# problem constants
B, C, T, HH, WW = 16, 64, 16, 36, 36
N, D = 4096, C * T             # 4096 neurons, 1024 input dim
P = HH * WW                    # 1296 flat positions
NCORES = 8
NBG = 4                        # batch groups
NDH = 2                        # D halves
BPC = B // NBG                 # batches per core = 4
DH = D // NDH                  # channels per core = 512
NC2 = DH // 256                # 2 double-subtile (256-chan) passes per core
PAD = 38                       # max corner offset (37) + 1
WINMAX = 128                   # psum bank: BPC*WIN <= 512 fp32
FEATW = P + PAD                # padded feat width per (chunk, batch)
GRPN = 4                       # blocks per W DMA group

F32 = mybir.dt.float32

import ml_dtypes
F8_DT = mybir.dt.float8e4
F8_NP = ml_dtypes.float8_e4m3   # max normal 240
F8_CAP = np.float32(224.0)


def _split_waits(nc, max_waits=1):
    """Walrus in this image allows only ONE sem wait per instruction.
    Hoist extra waits onto injected same-engine NoOps placed immediately
    before the owning instruction (same engine + program order => same
    semantics)."""
    k = 0
    for fn in nc.m.functions:
        for blk in fn.blocks:
            insts = blk.instructions
            out = []
            for inst in insts:
                si = inst.sync_info
                if si is not None and si.on_wait and len(si.on_wait) > max_waits:
                    waits = list(si.on_wait)
                    for w in waits[:-max_waits]:
                        nop = mybir.InstNoOp(name=f"I-wsplit-{k}", ins=[], outs=[])
                        k += 1
                        nop.engine = inst.engine
                        nop.sync_info = bass_rust.SyncInfo(
                            on_wait=[w], on_update=[]
                        )
                        out.append(nop)
                    si.on_wait = waits[-max_waits:]
                    inst.sync_info = si
                out.append(inst)
            if len(out) != len(insts):
                insts.clear()
                insts.extend(out)


def _bilinear_tables(mu):
    """Per-neuron base cell p00, corner offsets (4) in {0,1,36,37}, corner
    weights (4), replicating reference float32 arithmetic exactly."""
    one, half = np.float32(1.0), np.float32(0.5)
    g = np.clip(mu.astype(np.float32), -one, one)
    ix = (g[:, 0] + one) * np.float32(WW * 0.5) - half
    iy = (g[:, 1] + one) * np.float32(HH * 0.5) - half
    x0 = np.floor(ix)
    y0 = np.floor(iy)
    wx1 = ix - x0
    wx0 = one - wx1
    wy1 = iy - y0
    wy0 = one - wy1

    xs = [x0, x0 + one]
    ys = [y0, y0 + one]
    wxs = [wx0, wx1]
    wys = [wy0, wy1]

    x0c = np.clip(x0, 0, WW - 1).astype(np.int64)
    y0c = np.clip(y0, 0, HH - 1).astype(np.int64)
    p00 = y0c * WW + x0c

    offs = np.zeros((4, N), np.int64)
    wgts = np.zeros((4, N), np.float32)
    k = 0
    for a in range(2):          # y corner
        for bb in range(2):     # x corner
            xx, yy = xs[bb], ys[a]
            valid = (xx >= 0) & (xx <= WW - 1) & (yy >= 0) & (yy <= HH - 1)
            xi = np.clip(xx, 0, WW - 1).astype(np.int64)
            yi = np.clip(yy, 0, HH - 1).astype(np.int64)
            offs[k] = yi * WW + xi - p00
            wgts[k] = (wys[a] * wxs[bb]) * valid.astype(np.float32)
            k += 1
    assert offs.min() >= 0 and offs.max() <= 37
    return p00, offs, wgts


def _make_blocks(p00_sorted):
    """Greedy blocks of <=128 sorted neurons with window <= WINMAX."""
    blocks = []  # (start, end) into sorted order
    s = 0
    n = len(p00_sorted)
    while s < n:
        pfirst = p00_sorted[s]
        e = s
        while e < n and e - s < 128 and (p00_sorted[e] - pfirst) + PAD <= WINMAX:
            e += 1
        blocks.append((s, e))
        s = e
    return blocks


def kernel(x, mu, sigma, W, b):
    x = np.ascontiguousarray(x, dtype=np.float32)
    W = np.ascontiguousarray(W, dtype=np.float32)
    b = np.asarray(b, dtype=np.float32)

    p00, offs, wgts = _bilinear_tables(mu)
    order = np.argsort(p00, kind="stable")
    p00s = p00[order]
    blocks = _make_blocks(p00s)
    nblk = len(blocks)

    # ---- fp8 quantization: global x scale, per-neuron W scale ----
    sx = F8_CAP / np.float32(max(np.abs(x).max(), 1e-30))
    sw = F8_CAP / np.maximum(np.abs(W).max(axis=1), 1e-30).astype(np.float32)
    Wq = (W * sw[:, None]).astype(F8_NP)    # [N, D]
    dequant = 1.0 / (sw * sx)               # [N] folded into the mask

    # per-block host data
    wins, pfirsts, ms, sblk = [], [], [], []
    mparts = []
    for i, (s, e) in enumerate(blocks):
        idx = order[s:e]
        m = e - s
        pfirst = int(p00s[s])
        win = int(p00s[e - 1]) - pfirst + PAD
        ms.append(m)
        pfirsts.append(pfirst)
        wins.append(win)
        sblk.append(s)
        # mask [128, win], fp8 dequant folded in
        mk = np.zeros((128, win), np.float32)
        rel = (p00[idx] - pfirst)  # [m]
        for k in range(4):
            np.add.at(mk[:m], (np.arange(m), rel + offs[k][idx]),
                      wgts[k][idx] * dequant[idx])
        mparts.append(mk)
    sblk.append(N)

    # W groups: first small so the PE can start early, the rest sized GRPN
    gbounds = [0, min(2, nblk)]
    while gbounds[-1] + GRPN < nblk:
        gbounds.append(gbounds[-1] + GRPN)
    if gbounds[-1] < nblk:
        gbounds.append(nblk)
    ngrp = len(gbounds) - 1

    # W packed per group with one contiguous row per partition:
    # group layout [128, NC2, 2, sum_m(group)]; groups concatenated flat.
    Ws = Wq[order]                          # [N, D] sorted
    gw_off = [0]
    for g in range(ngrp):
        gm = sblk[gbounds[g + 1]] - sblk[gbounds[g]]
        gw_off.append(gw_off[-1] + NC2 * 2 * gm)
    walls = []
    for dh in range(NDH):
        wl = (Ws[:, dh * DH:(dh + 1) * DH].T        # [512, N]
              .reshape(NC2, 2, 128, N).transpose(2, 0, 1, 3))  # [128,NC2,2,N]
        parts = []
        for g in range(ngrp):
            lo, hi = sblk[gbounds[g]], sblk[gbounds[g + 1]]
            parts.append(wl[:, :, :, lo:hi].reshape(128, -1))
        walls.append(np.ascontiguousarray(np.concatenate(parts, axis=1)))
    mask_all = np.ascontiguousarray(
        np.concatenate(mparts, axis=1)).astype(ml_dtypes.bfloat16)
    moffs = np.cumsum([0] + [w for w in wins])
    mtot = int(mask_all.shape[1])

    # ---- build the Bass program (same for all cores) ----
    nc = bass.Bass()
    xs_h = nc.declare_dram_parameter("xs", [128, NC2, 2, BPC, FEATW], F8_DT,
                                     isOutput=False)
    wf_h = nc.declare_dram_parameter("wf", [128, int(gw_off[-1])], F8_DT,
                                     isOutput=False)
    mf_h = nc.declare_dram_parameter("mf", [128, mtot], mybir.dt.bfloat16,
                                    isOutput=False)
    z_h = nc.declare_dram_parameter("z", [128, BPC * nblk], F32, isOutput=True)

    ADD = mybir.AluOpType.add
    MULT = mybir.AluOpType.mult
    DR = mybir.MatmulPerfMode.DoubleRow

    with tile.TileContext(nc) as tc:
        with (
            tc.tile_pool(name="feat", bufs=1) as featp,
            tc.tile_pool(name="wpool", bufs=1) as wpool,
            tc.tile_pool(name="mpool", bufs=1) as mpool,
            tc.tile_pool(name="spool", bufs=4) as spool,
            tc.tile_pool(name="zpool", bufs=1) as zpool,
            tc.tile_pool(name="psum", bufs=1, space="PSUM") as psump,
        ):
            fts = [featp.tile([128, 2, BPC, FEATW], F8_DT, name=f"feat{c}")
                   for c in range(NC2)]
            mask_t = mpool.tile([128, mtot], mybir.dt.bfloat16)
            zAll = zpool.tile([128, BPC * nblk], F32)
            wgs = {}
            for g in range(ngrp):
                gcols = sblk[gbounds[g + 1]] - sblk[gbounds[g]]
                wgs[g] = wpool.tile([128, NC2, 2, gcols], F8_DT, name=f"wg{g}")

            def wg_item(g):
                return (wgs[g][:], wf_h[:, int(gw_off[g]):int(gw_off[g + 1])])

            def mask_item(g):
                lo = int(moffs[gbounds[g]])
                hi = int(moffs[gbounds[g + 1]])
                return (mask_t[:, lo:hi], mf_h[:, lo:hi])

            # One contiguous DMA per feat double-chunk (10.7KB rows, full
            # DMA rate), one per queue in parallel; then W0+mask0 split
            # across both queues (earliest PE start), then the remaining
            # groups + mask slices alternating queues need-ordered.
            # One contiguous DMA per feat double-chunk (10.7KB rows, full
            # DMA rate), one per queue in parallel; W0 halves right after
            # on both queues; then remaining groups + masks alternating.
            g0cols = sblk[gbounds[1]] - sblk[gbounds[0]]
            g0h = 2 * g0cols  # elements per cc2 slice of group 0
            sync_items = [[(fts[0][:], xs_h[:, 0])],
                          [(wgs[0][:, 0], wf_h[:, 0:g0h]), mask_item(0)]]
            scal_items = [[(fts[1][:], xs_h[:, 1])],
                          [(wgs[0][:, 1], wf_h[:, g0h:2 * g0h])]]
            for g in range(1, ngrp):
                (sync_items if (g % 2 == 0) else scal_items).append(
                    [wg_item(g), mask_item(g)]
                )
            plan = {nc.sync: sync_items, nc.scalar: scal_items}
            maxlen = max(len(v) for v in plan.values())
            for k in range(maxlen):
                for eng, items in plan.items():
                    if k < len(items):
                        for dst, srcap in items[k]:
                            eng.dma_start(dst, srcap)

            for g in range(ngrp):
                blks = list(range(gbounds[g], gbounds[g + 1]))
                wg = wgs[g]
                glo = sblk[gbounds[g]]
                pms = {}
                for i in blks:
                    pms[i] = psump.tile([128, BPC, wins[i]], F32,
                                        name=f"pm{i}", tag=f"pm{i % 8}")
                for c in range(NC2):
                    for i in blks:
                        m, win, pfirst = ms[i], wins[i], pfirsts[i]
                        o = sblk[i] - glo
                        nc.tensor.matmul(
                            pms[i][0:m, :, :],
                            wg[:, c, :, o:o + m],
                            fts[c][:, :, :, pfirst:pfirst + win],
                            start=(c == 0),
                            stop=(c == NC2 - 1),
                            perf_mode=DR,
                        )
                # fused mask-mult + window-reduce on DVE (the only engine
                # that can do tensor*tensor reads from PSUM; offloading the
                # reduce to Act still leaves DVE paying the PSUM-read
                # multiply, so a split buys nothing)
                for i in blks:
                    m, win = ms[i], wins[i]
                    mo = int(moffs[i])
                    for bb in range(BPC):
                        sc = spool.tile([128, WINMAX], F32, tag=f"sv{bb}")
                        nc.vector.scalar_tensor_tensor(
                            sc[0:m, 0:win],
                            pms[i][0:m, bb, :],
                            0.0,
                            mask_t[0:m, mo:mo + win],
                            ADD,
                            MULT,
                            accum_out=zAll[0:m, BPC * i + bb:BPC * i + bb + 1],
                        )
                # store this group's partials right away (overlaps the
                # rest); the final group stores per block so the very last
                # DMA waits only on the last block's reduce
                if g == ngrp - 1:
                    for i in blks:
                        sl = slice(BPC * i, BPC * (i + 1))
                        nc.sync.dma_start(z_h[:, sl], zAll[:, sl])
                else:
                    sl = slice(BPC * gbounds[g], BPC * gbounds[g + 1])
                    nc.sync.dma_start(z_h[:, sl], zAll[:, sl])

    _split_waits(nc)

    # ---- run on 8 cores: core id = bg*2 + dh ----
    xq = (x.reshape(B, D // 128, 128, P) * sx).astype(F8_NP)
    in_maps = []
    for core in range(NCORES):
        bg, dh = core // NDH, core % NDH
        xs_dev = np.zeros((128, NC2, 2, BPC, FEATW), F8_NP)
        # xq[b, chunk128, p, pos] -> [p, cc2, i, bb, pos]
        blkx = xq[BPC * bg:BPC * (bg + 1),
                  4 * dh:4 * (dh + 1)].reshape(BPC, NC2, 2, 128, P)
        xs_dev[:, :, :, :, :P] = blkx.transpose(3, 1, 2, 0, 4)
        in_maps.append({
            "xs": xs_dev,
            "wf": walls[dh],
            "mf": mask_all,
        })
    res = run_bass_kernel_spmd(nc, in_maps, core_ids=list(range(NCORES)))

    # ---- assemble: add D-halves, bias, elu(y)+1 ----
    y = np.empty((B, N), np.float32)
    for bg in range(NBG):
        z = res.results[NDH * bg]["z"] + res.results[NDH * bg + 1]["z"]
        for i, (s, e) in enumerate(blocks):
            idx = order[s:e]
            m = e - s
            y[BPC * bg:BPC * (bg + 1), idx] = z[0:m, BPC * i:BPC * (i + 1)].T
    y += b
    return np.where(y > 0, y + np.float32(1.0),
                    np.exp(np.minimum(y, np.float32(0.0)))).astype(np.float32)


# revision 35
# speedup vs baseline: 1.0205x; 1.0205x over previous
"""PoissonGaussianReadout forward on 8 trn2 NeuronCores.

Math (eval mode): each neuron n samples feat[b] (a [36,36,1024] image per
batch, 1024 = C*T channels) bilinearly at a fixed point mu[n], then takes a
per-neuron dot with W[n,:], adds b[n], applies elu(y)+1.

Strategy:
  - Hybrid shard 4x2: 8 cores = 4 batch-groups (4 batches each) x 2 halves
    of the contraction dim D (512 channels each).  Cores emit LINEAR
    partial sums; the host adds the halves, bias, and elu on [16,4096].
    (Splitting D halves the per-core W traffic; splitting batches keeps
    the per-core feat traffic at 1/8 -- together they minimize both the
    DMA stream and the feat-arrival gate.)
  - fp8(e4m3) x and W with DoubleRow matmuls: x uses one global scale, W a
    per-neuron scale; both dequant factors fold into the (per-neuron) mask.
    Halves both the DMA stream and the PE time vs bf16 (rel err 1.3e-2,
    within the 2e-2 gate; inputs are deterministic).
  - Sort neurons by bilinear base cell p00 = y0*36+x0; blocks of <=128
    sorted neurons span a window of <=WINMAX flat positions.  Two
    DoubleRow matmuls per block (256-channel subtile pairs):
    psum[n, (b,j)] += Wblk^T @ feat-window, a contiguous slice of feat.
  - Each neuron's 4 bilinear corners live at window offsets
    (p00-pfirst)+{0,1,36,37}; a host-built sparse mask [n, win] (bf16,
    fp8 dequant folded in) holds the bilinear weights.  One DVE
    scalar_tensor_tensor per (block, batch) fuses mask-multiply and
    window-reduce straight out of PSUM into z.  This DVE phase (~21us)
    is the critical path: it is the only engine that can do
    tensor*tensor reads from PSUM, and its cost is bound by
    per-instruction window elements + accumulator drains.
  - DMA is need-ordered on the two HWDGE queues: one contiguous feat
    double-chunk per queue first (the whole PE/DVE pipeline gates on
    feat), then W block-groups + their mask slices just-in-time behind
    PE consumption; per-group z stores overlap the pipeline.
"""
import sys
sys.path.insert(0, "/opt/trn_rl_repo")

import numpy as np

from concourse import bass, mybir, tile
from concourse.bass_utils import run_bass_kernel_spmd
import bass_rust

# problem constants
B, C, T, HH, WW = 16, 64, 16, 36, 36
N, D = 4096, C * T             # 4096 neurons, 1024 input dim
P = HH * WW                    # 1296 flat positions
NCORES = 8
NBG = 4                        # batch groups
NDH = 2                        # D halves
BPC = B // NBG                 # batches per core = 4
DH = D // NDH                  # channels per core = 512
NC2 = DH // 256                # 2 double-subtile (256-chan) passes per core
PAD = 38                       # max corner offset (37) + 1
WINMAX = 128                   # psum bank: BPC*WIN <= 512 fp32
FEATW = P + PAD                # padded feat width per (chunk, batch)
GRPN = 4                       # blocks per W DMA group

F32 = mybir.dt.float32

import ml_dtypes
F8_DT = mybir.dt.float8e4
F8_NP = ml_dtypes.float8_e4m3   # max normal 240
F8_CAP = np.float32(224.0)


def _split_waits(nc, max_waits=1):
    """Walrus in this image allows only ONE sem wait per instruction.
    Hoist extra waits onto injected same-engine NoOps placed immediately
    before the owning instruction (same engine + program order => same
    semantics)."""
    k = 0
    for fn in nc.m.functions:
        for blk in fn.blocks:
            insts = blk.instructions
            out = []
            for inst in insts:
                si = inst.sync_info
                if si is not None and si.on_wait and len(si.on_wait) > max_waits:
                    waits = list(si.on_wait)
                    for w in waits[:-max_waits]:
                        nop = mybir.InstNoOp(name=f"I-wsplit-{k}", ins=[], outs=[])
                        k += 1
                        nop.engine = inst.engine
                        nop.sync_info = bass_rust.SyncInfo(
                            on_wait=[w], on_update=[]
                        )
                        out.append(nop)
                    si.on_wait = waits[-max_waits:]
                    inst.sync_info = si
                out.append(inst)
            if len(out) != len(insts):
                insts.clear()
                insts.extend(out)


def _bilinear_tables(mu):
    """Per-neuron base cell p00, corner offsets (4) in {0,1,36,37}, corner
    weights (4), replicating reference float32 arithmetic exactly."""
    one, half = np.float32(1.0), np.float32(0.5)
    g = np.clip(mu.astype(np.float32), -one, one)
    ix = (g[:, 0] + one) * np.float32(WW * 0.5) - half
    iy = (g[:, 1] + one) * np.float32(HH * 0.5) - half
    x0 = np.floor(ix)
    y0 = np.floor(iy)
    wx1 = ix - x0
    wx0 = one - wx1
    wy1 = iy - y0
    wy0 = one - wy1

    xs = [x0, x0 + one]
    ys = [y0, y0 + one]
    wxs = [wx0, wx1]
    wys = [wy0, wy1]

    x0c = np.clip(x0, 0, WW - 1).astype(np.int64)
    y0c = np.clip(y0, 0, HH - 1).astype(np.int64)
    p00 = y0c * WW + x0c

    offs = np.zeros((4, N), np.int64)
    wgts = np.zeros((4, N), np.float32)
    k = 0
    for a in range(2):          # y corner
        for bb in range(2):     # x corner
            xx, yy = xs[bb], ys[a]
            valid = (xx >= 0) & (xx <= WW - 1) & (yy >= 0) & (yy <= HH - 1)
            xi = np.clip(xx, 0, WW - 1).astype(np.int64)
            yi = np.clip(yy, 0, HH - 1).astype(np.int64)
            offs[k] = yi * WW + xi - p00
            wgts[k] = (wys[a] * wxs[bb]) * valid.astype(np.float32)
            k += 1
    assert offs.min() >= 0 and offs.max() <= 37
    return p00, offs, wgts


def _make_blocks(p00_sorted):
    """Greedy blocks of <=128 sorted neurons with window <= WINMAX."""
    blocks = []  # (start, end) into sorted order
    s = 0
    n = len(p00_sorted)
    while s < n:
        pfirst = p00_sorted[s]
        e = s
        while e < n and e - s < 128 and (p00_sorted[e] - pfirst) + PAD <= WINMAX:
            e += 1
        blocks.append((s, e))
        s = e
    return blocks


def kernel(x, mu, sigma, W, b):
    x = np.ascontiguousarray(x, dtype=np.float32)
    W = np.ascontiguousarray(W, dtype=np.float32)
    b = np.asarray(b, dtype=np.float32)

    p00, offs, wgts = _bilinear_tables(mu)
    order = np.argsort(p00, kind="stable")
    p00s = p00[order]
    blocks = _make_blocks(p00s)
    nblk = len(blocks)

    # ---- fp8 quantization: global x scale, per-neuron W scale ----
    sx = F8_CAP / np.float32(max(np.abs(x).max(), 1e-30))
    sw = F8_CAP / np.maximum(np.abs(W).max(axis=1), 1e-30).astype(np.float32)
    Wq = (W * sw[:, None]).astype(F8_NP)    # [N, D]
    dequant = 1.0 / (sw * sx)               # [N] folded into the mask

    # per-block host data
    wins, pfirsts, ms, sblk = [], [], [], []
    mparts = []
    for i, (s, e) in enumerate(blocks):
        idx = order[s:e]
        m = e - s
        pfirst = int(p00s[s])
        win = int(p00s[e - 1]) - pfirst + PAD
        ms.append(m)
        pfirsts.append(pfirst)
        wins.append(win)
        sblk.append(s)
        # mask [128, win], fp8 dequant folded in
        mk = np.zeros((128, win), np.float32)
        rel = (p00[idx] - pfirst)  # [m]
        for k in range(4):
            np.add.at(mk[:m], (np.arange(m), rel + offs[k][idx]),
                      wgts[k][idx] * dequant[idx])
        mparts.append(mk)
    sblk.append(N)

    # W groups: first small so the PE can start early, the rest sized GRPN
    gbounds = [0, min(2, nblk)]
    while gbounds[-1] + GRPN < nblk:
        gbounds.append(gbounds[-1] + GRPN)
    if gbounds[-1] < nblk:
        gbounds.append(nblk)
    ngrp = len(gbounds) - 1

    # W packed per group with one contiguous row per partition:
    # group layout [128, NC2, 2, sum_m(group)]; groups concatenated flat.
    Ws = Wq[order]                          # [N, D] sorted
    gw_off = [0]
    for g in range(ngrp):
        gm = sblk[gbounds[g + 1]] - sblk[gbounds[g]]
        gw_off.append(gw_off[-1] + NC2 * 2 * gm)
    walls = []
    for dh in range(NDH):
        wl = (Ws[:, dh * DH:(dh + 1) * DH].T        # [512, N]
              .reshape(NC2, 2, 128, N).transpose(2, 0, 1, 3))  # [128,NC2,2,N]
        parts = []
        for g in range(ngrp):
            lo, hi = sblk[gbounds[g]], sblk[gbounds[g + 1]]
            parts.append(wl[:, :, :, lo:hi].reshape(128, -1))
        walls.append(np.ascontiguousarray(np.concatenate(parts, axis=1)))
    mask_all = np.ascontiguousarray(
        np.concatenate(mparts, axis=1)).astype(ml_dtypes.bfloat16)
    moffs = np.cumsum([0] + [w for w in wins])
    mtot = int(mask_all.shape[1])

    # ---- build the Bass program (same for all cores) ----
    nc = bass.Bass()
    xs_h = nc.declare_dram_parameter("xs", [128, NC2, 2, BPC, FEATW], F8_DT,
                                     isOutput=False)
    wf_h = nc.declare_dram_parameter("wf", [128, int(gw_off[-1])], F8_DT,
                                     isOutput=False)
    mf_h = nc.declare_dram_parameter("mf", [128, mtot], mybir.dt.bfloat16,
                                    isOutput=False)
    z_h = nc.declare_dram_parameter("z", [128, BPC * nblk], F32, isOutput=True)

    ADD = mybir.AluOpType.add
    MULT = mybir.AluOpType.mult
    DR = mybir.MatmulPerfMode.DoubleRow

    with tile.TileContext(nc) as tc:
        with (
            tc.tile_pool(name="feat", bufs=1) as featp,
            tc.tile_pool(name="wpool", bufs=1) as wpool,
            tc.tile_pool(name="mpool", bufs=1) as mpool,
            tc.tile_pool(name="spool", bufs=4) as spool,
            tc.tile_pool(name="zpool", bufs=1) as zpool,
            tc.tile_pool(name="psum", bufs=1, space="PSUM") as psump,
        ):
            fts = [featp.tile([128, 2, BPC, FEATW], F8_DT, name=f"feat{c}")
                   for c in range(NC2)]
            mask_t = mpool.tile([128, mtot], mybir.dt.bfloat16)
            zAll = zpool.tile([128, BPC * nblk], F32)
            wgs = {}
            for g in range(ngrp):
                gcols = sblk[gbounds[g + 1]] - sblk[gbounds[g]]
                wgs[g] = wpool.tile([128, NC2, 2, gcols], F8_DT, name=f"wg{g}")

            def wg_item(g):
                return (wgs[g][:], wf_h[:, int(gw_off[g]):int(gw_off[g + 1])])

            def mask_item(g):
                lo = int(moffs[gbounds[g]])
                hi = int(moffs[gbounds[g + 1]])
                return (mask_t[:, lo:hi], mf_h[:, lo:hi])

            # One contiguous DMA per feat double-chunk (10.7KB rows, full
            # DMA rate), one per queue in parallel; then W0+mask0 split
            # across both queues (earliest PE start), then the remaining
            # groups + mask slices alternating queues need-ordered.
            # One contiguous DMA per feat double-chunk (10.7KB rows, full
            # DMA rate), one per queue in parallel; W0 halves right after
            # on both queues; then remaining groups + masks alternating.
            g0cols = sblk[gbounds[1]] - sblk[gbounds[0]]
            g0h = 2 * g0cols  # elements per cc2 slice of group 0
            sync_items = [[(fts[0][:], xs_h[:, 0])],
                          [(wgs[0][:, 0], wf_h[:, 0:g0h]), mask_item(0)]]
            scal_items = [[(fts[1][:], xs_h[:, 1])],
                          [(wgs[0][:, 1], wf_h[:, g0h:2 * g0h])]]
            for g in range(1, ngrp):
                (sync_items if (g % 2 == 0) else scal_items).append(
                    [wg_item(g), mask_item(g)]
                )
            plan = {nc.sync: sync_items, nc.scalar: scal_items}
            maxlen = max(len(v) for v in plan.values())
            for k in range(maxlen):
                for eng, items in plan.items():
                    if k < len(items):
                        for dst, srcap in items[k]:
                            eng.dma_start(dst, srcap)

            for g in range(ngrp):
                blks = list(range(gbounds[g], gbounds[g + 1]))
                wg = wgs[g]
                glo = sblk[gbounds[g]]
                pms = {}
                for i in blks:
                    pms[i] = psump.tile([128, BPC, wins[i]], F32,
                                        name=f"pm{i}", tag=f"pm{i % 8}")
                for c in range(NC2):
                    for i in blks:
                        m, win, pfirst = ms[i], wins[i], pfirsts[i]
                        o = sblk[i] - glo
                        nc.tensor.matmul(
                            pms[i][0:m, :, :],
                            wg[:, c, :, o:o + m],
                            fts[c][:, :, :, pfirst:pfirst + win],
                            start=(c == 0),
                            stop=(c == NC2 - 1),
                            perf_mode=DR,
                        )
                # fused mask-mult + window-reduce on DVE (the only engine
                # that can do tensor*tensor reads from PSUM; offloading the
                # reduce to Act still leaves DVE paying the PSUM-read
                # multiply, so a split buys nothing)
                for i in blks:
                    m, win = ms[i], wins[i]
                    mo = int(moffs[i])
                    for bb in range(BPC):
                        sc = spool.tile([128, WINMAX], F32, tag=f"sv{bb}")
                        nc.vector.scalar_tensor_tensor(
                            sc[0:m, 0:win],
                            pms[i][0:m, bb, :],
                            0.0,
                            mask_t[0:m, mo:mo + win],
                            ADD,
                            MULT,
                            accum_out=zAll[0:m, BPC * i + bb:BPC * i + bb + 1],
                        )
                pass
            # Single output store at the end: per-group stores look like
            # better overlap but the store's READ of zAll adds a WAR edge
            # that stalls every later accumulate into zAll.
            nc.sync.dma_start(z_h[:], zAll[:])

    _split_waits(nc)

    # ---- run on 8 cores: core id = bg*2 + dh ----
    xq = (x.reshape(B, D // 128, 128, P) * sx).astype(F8_NP)
    in_maps = []
    for core in range(NCORES):
        bg, dh = core // NDH, core % NDH
        xs_dev = np.zeros((128, NC2, 2, BPC, FEATW), F8_NP)
        # xq[b, chunk128, p, pos] -> [p, cc2, i, bb, pos]
        blkx = xq[BPC * bg:BPC * (bg + 1),
                  4 * dh:4 * (dh + 1)].reshape(BPC, NC2, 2, 128, P)
        xs_dev[:, :, :, :, :P] = blkx.transpose(3, 1, 2, 0, 4)
        in_maps.append({
            "xs": xs_dev,
            "wf": walls[dh],
            "mf": mask_all,
        })
    res = run_bass_kernel_spmd(nc, in_maps, core_ids=list(range(NCORES)))

    # ---- assemble: add D-halves, bias, elu(y)+1 ----
    y = np.empty((B, N), np.float32)
    for bg in range(NBG):
        z = res.results[NDH * bg]["z"] + res.results[NDH * bg + 1]["z"]
        for i, (s, e) in enumerate(blocks):
            idx = order[s:e]
            m = e - s
            y[BPC * bg:BPC * (bg + 1), idx] = z[0:m, BPC * i:BPC * (i + 1)].T
    y += b
    return np.where(y > 0, y + np.float32(1.0),
                    np.exp(np.minimum(y, np.float32(0.0)))).astype(np.float32)


# revision 36
# speedup vs baseline: 1.1292x; 1.1065x over previous
"""PoissonGaussianReadout forward on 8 trn2 NeuronCores.

Math (eval mode): each neuron n samples feat[b] (a [36,36,1024] image per
batch, 1024 = C*T channels) bilinearly at a fixed point mu[n], then takes a
per-neuron dot with W[n,:], adds b[n], applies elu(y)+1.

Strategy:
  - Hybrid shard 4x2: 8 cores = 4 batch-groups (4 batches each) x 2 halves
    of the contraction dim D (512 channels each).  Cores emit LINEAR
    partial sums; the host adds the halves, bias, and elu on [16,4096].
    (Splitting D halves the per-core W traffic; splitting batches keeps
    the per-core feat traffic at 1/8 -- together they minimize both the
    DMA stream and the feat-arrival gate.)
  - fp8(e4m3) x and W with DoubleRow matmuls: x uses one global scale, W a
    per-neuron scale; both dequant factors fold into the (per-neuron) mask.
    Halves both the DMA stream and the PE time vs bf16 (rel err 1.3e-2,
    within the 2e-2 gate; inputs are deterministic).
  - Sort neurons by bilinear base cell p00 = y0*36+x0; blocks of <=128
    sorted neurons span a window of <=WINMAX flat positions.  Two
    DoubleRow matmuls per block (256-channel subtile pairs):
    psum[n, (b,j)] += Wblk^T @ feat-window, a contiguous slice of feat.
  - Each neuron's 4 bilinear corners live at window offsets
    (p00-pfirst)+{0,1,36,37}; a host-built sparse mask [n, win] (bf16,
    fp8 dequant folded in) holds the bilinear weights.  One DVE
    scalar_tensor_tensor per (block, batch) fuses mask-multiply and
    window-reduce straight out of PSUM into z.  This DVE phase (~21us)
    is the critical path: it is the only engine that can do
    tensor*tensor reads from PSUM, and its cost is bound by
    per-instruction window elements + accumulator drains.
  - DMA is need-ordered on the two HWDGE queues: one contiguous feat
    double-chunk per queue first (the whole PE/DVE pipeline gates on
    feat), then W block-groups + their mask slices just-in-time behind
    PE consumption; per-group z stores overlap the pipeline.
"""
import sys
sys.path.insert(0, "/opt/trn_rl_repo")

import numpy as np

from concourse import bass, mybir, tile
from concourse.bass_utils import run_bass_kernel_spmd
import bass_rust

# problem constants
B, C, T, HH, WW = 16, 64, 16, 36, 36
N, D = 4096, C * T             # 4096 neurons, 1024 input dim
P = HH * WW                    # 1296 flat positions
NCORES = 8
NBG = 4                        # batch groups
NDH = 2                        # D halves
BPC = B // NBG                 # batches per core = 4
DH = D // NDH                  # channels per core = 512
NC2 = DH // 256                # 2 double-subtile (256-chan) passes per core
PAD = 38                       # max corner offset (37) + 1
WINMAX = 128                   # psum bank: BPC*WIN <= 512 fp32
FEATW = P + PAD                # padded feat width per (chunk, batch)
GRPN = 4                       # blocks per W DMA group

F32 = mybir.dt.float32

import ml_dtypes
F8_DT = mybir.dt.float8e4
F8_NP = ml_dtypes.float8_e4m3   # max normal 240
F8_CAP = np.float32(224.0)


def _split_waits(nc, max_waits=1):
    """Walrus in this image allows only ONE sem wait per instruction.
    Hoist extra waits onto injected same-engine NoOps placed immediately
    before the owning instruction (same engine + program order => same
    semantics)."""
    k = 0
    for fn in nc.m.functions:
        for blk in fn.blocks:
            insts = blk.instructions
            out = []
            for inst in insts:
                si = inst.sync_info
                if si is not None and si.on_wait and len(si.on_wait) > max_waits:
                    waits = list(si.on_wait)
                    for w in waits[:-max_waits]:
                        nop = mybir.InstNoOp(name=f"I-wsplit-{k}", ins=[], outs=[])
                        k += 1
                        nop.engine = inst.engine
                        nop.sync_info = bass_rust.SyncInfo(
                            on_wait=[w], on_update=[]
                        )
                        out.append(nop)
                    si.on_wait = waits[-max_waits:]
                    inst.sync_info = si
                out.append(inst)
            if len(out) != len(insts):
                insts.clear()
                insts.extend(out)


def _bilinear_tables(mu):
    """Per-neuron base cell p00, corner offsets (4) in {0,1,36,37}, corner
    weights (4), replicating reference float32 arithmetic exactly."""
    one, half = np.float32(1.0), np.float32(0.5)
    g = np.clip(mu.astype(np.float32), -one, one)
    ix = (g[:, 0] + one) * np.float32(WW * 0.5) - half
    iy = (g[:, 1] + one) * np.float32(HH * 0.5) - half
    x0 = np.floor(ix)
    y0 = np.floor(iy)
    wx1 = ix - x0
    wx0 = one - wx1
    wy1 = iy - y0
    wy0 = one - wy1

    xs = [x0, x0 + one]
    ys = [y0, y0 + one]
    wxs = [wx0, wx1]
    wys = [wy0, wy1]

    x0c = np.clip(x0, 0, WW - 1).astype(np.int64)
    y0c = np.clip(y0, 0, HH - 1).astype(np.int64)
    p00 = y0c * WW + x0c

    offs = np.zeros((4, N), np.int64)
    wgts = np.zeros((4, N), np.float32)
    k = 0
    for a in range(2):          # y corner
        for bb in range(2):     # x corner
            xx, yy = xs[bb], ys[a]
            valid = (xx >= 0) & (xx <= WW - 1) & (yy >= 0) & (yy <= HH - 1)
            xi = np.clip(xx, 0, WW - 1).astype(np.int64)
            yi = np.clip(yy, 0, HH - 1).astype(np.int64)
            offs[k] = yi * WW + xi - p00
            wgts[k] = (wys[a] * wxs[bb]) * valid.astype(np.float32)
            k += 1
    assert offs.min() >= 0 and offs.max() <= 37
    return p00, offs, wgts


def _make_blocks(p00_sorted):
    """Greedy blocks of <=128 sorted neurons with window <= WINMAX."""
    blocks = []  # (start, end) into sorted order
    s = 0
    n = len(p00_sorted)
    while s < n:
        pfirst = p00_sorted[s]
        e = s
        while e < n and e - s < 128 and (p00_sorted[e] - pfirst) + PAD <= WINMAX:
            e += 1
        blocks.append((s, e))
        s = e
    return blocks


def kernel(x, mu, sigma, W, b):
    x = np.ascontiguousarray(x, dtype=np.float32)
    W = np.ascontiguousarray(W, dtype=np.float32)
    b = np.asarray(b, dtype=np.float32)

    p00, offs, wgts = _bilinear_tables(mu)
    order = np.argsort(p00, kind="stable")
    p00s = p00[order]
    blocks = _make_blocks(p00s)
    nblk = len(blocks)

    # ---- fp8 quantization: global x scale, per-neuron W scale ----
    sx = F8_CAP / np.float32(max(np.abs(x).max(), 1e-30))
    sw = F8_CAP / np.maximum(np.abs(W).max(axis=1), 1e-30).astype(np.float32)
    Wq = (W * sw[:, None]).astype(F8_NP)    # [N, D]
    dequant = 1.0 / (sw * sx)               # [N] folded into the mask

    # per-block host data
    wins, pfirsts, ms, sblk = [], [], [], []
    mparts = []
    for i, (s, e) in enumerate(blocks):
        idx = order[s:e]
        m = e - s
        pfirst = int(p00s[s])
        win = int(p00s[e - 1]) - pfirst + PAD
        ms.append(m)
        pfirsts.append(pfirst)
        wins.append(win)
        sblk.append(s)
        # mask [128, win], fp8 dequant folded in
        mk = np.zeros((128, win), np.float32)
        rel = (p00[idx] - pfirst)  # [m]
        for k in range(4):
            np.add.at(mk[:m], (np.arange(m), rel + offs[k][idx]),
                      wgts[k][idx] * dequant[idx])
        mparts.append(mk)
    sblk.append(N)

    # W groups: first small so the PE can start early, the rest sized GRPN
    gbounds = [0, min(2, nblk)]
    while gbounds[-1] + GRPN < nblk:
        gbounds.append(gbounds[-1] + GRPN)
    if gbounds[-1] < nblk:
        gbounds.append(nblk)
    ngrp = len(gbounds) - 1

    # W packed per group with one contiguous row per partition:
    # group layout [128, NC2, 2, sum_m(group)]; groups concatenated flat.
    Ws = Wq[order]                          # [N, D] sorted
    gw_off = [0]
    for g in range(ngrp):
        gm = sblk[gbounds[g + 1]] - sblk[gbounds[g]]
        gw_off.append(gw_off[-1] + NC2 * 2 * gm)
    walls = []
    for dh in range(NDH):
        wl = (Ws[:, dh * DH:(dh + 1) * DH].T        # [512, N]
              .reshape(NC2, 2, 128, N).transpose(2, 0, 1, 3))  # [128,NC2,2,N]
        parts = []
        for g in range(ngrp):
            lo, hi = sblk[gbounds[g]], sblk[gbounds[g + 1]]
            parts.append(wl[:, :, :, lo:hi].reshape(128, -1))
        walls.append(np.ascontiguousarray(np.concatenate(parts, axis=1)))
    mask_all = np.ascontiguousarray(
        np.concatenate(mparts, axis=1)).astype(ml_dtypes.bfloat16)
    moffs = np.cumsum([0] + [w for w in wins])
    mtot = int(mask_all.shape[1])

    # ---- build the Bass program (same for all cores) ----
    nc = bass.Bass()
    xs_h = nc.declare_dram_parameter("xs", [128, NC2, 2, BPC, FEATW], F8_DT,
                                     isOutput=False)
    wf_h = nc.declare_dram_parameter("wf", [128, int(gw_off[-1])], F8_DT,
                                     isOutput=False)
    mf_h = nc.declare_dram_parameter("mf", [128, mtot], mybir.dt.bfloat16,
                                    isOutput=False)
    z_h = nc.declare_dram_parameter("z", [128, BPC * nblk], F32, isOutput=True)

    ADD = mybir.AluOpType.add
    MULT = mybir.AluOpType.mult
    DR = mybir.MatmulPerfMode.DoubleRow

    with tile.TileContext(nc) as tc:
        with (
            tc.tile_pool(name="feat", bufs=1) as featp,
            tc.tile_pool(name="wpool", bufs=1) as wpool,
            tc.tile_pool(name="mpool", bufs=1) as mpool,
            tc.tile_pool(name="spool", bufs=4) as spool,
            tc.tile_pool(name="zpool", bufs=1) as zpool,
            tc.tile_pool(name="psum", bufs=1, space="PSUM") as psump,
        ):
            fts = [featp.tile([128, 2, BPC, FEATW], F8_DT, name=f"feat{c}")
                   for c in range(NC2)]
            mask_t = mpool.tile([128, mtot], mybir.dt.bfloat16)
            zAll = zpool.tile([128, BPC * nblk], F32)
            wgs = {}
            for g in range(ngrp):
                gcols = sblk[gbounds[g + 1]] - sblk[gbounds[g]]
                wgs[g] = wpool.tile([128, NC2, 2, gcols], F8_DT, name=f"wg{g}")

            def wg_item(g):
                return (wgs[g][:], wf_h[:, int(gw_off[g]):int(gw_off[g + 1])])

            def mask_item(g):
                lo = int(moffs[gbounds[g]])
                hi = int(moffs[gbounds[g + 1]])
                return (mask_t[:, lo:hi], mf_h[:, lo:hi])

            # One contiguous DMA per feat double-chunk (10.7KB rows, full
            # DMA rate), one per queue in parallel; then W0+mask0 split
            # across both queues (earliest PE start), then the remaining
            # groups + mask slices alternating queues need-ordered.
            # One contiguous DMA per feat double-chunk (10.7KB rows, full
            # DMA rate), one per queue in parallel; W0 halves right after
            # on both queues; then remaining groups + masks alternating.
            g0cols = sblk[gbounds[1]] - sblk[gbounds[0]]
            g0h = 2 * g0cols  # elements per cc2 slice of group 0
            sync_items = [[(fts[0][:], xs_h[:, 0])],
                          [(wgs[0][:, 0], wf_h[:, 0:g0h]), mask_item(0)]]
            scal_items = [[(fts[1][:], xs_h[:, 1])],
                          [(wgs[0][:, 1], wf_h[:, g0h:2 * g0h])]]
            for g in range(1, ngrp):
                (sync_items if (g % 2 == 0) else scal_items).append(
                    [wg_item(g), mask_item(g)]
                )
            plan = {nc.sync: sync_items, nc.scalar: scal_items}
            maxlen = max(len(v) for v in plan.values())
            for k in range(maxlen):
                for eng, items in plan.items():
                    if k < len(items):
                        for dst, srcap in items[k]:
                            eng.dma_start(dst, srcap)

            for g in range(ngrp):
                blks = list(range(gbounds[g], gbounds[g + 1]))
                wg = wgs[g]
                glo = sblk[gbounds[g]]
                pms = {}
                for i in blks:
                    pms[i] = psump.tile([128, BPC, wins[i]], F32,
                                        name=f"pm{i}", tag=f"pm{i % 8}")
                for c in range(NC2):
                    for i in blks:
                        m, win, pfirst = ms[i], wins[i], pfirsts[i]
                        o = sblk[i] - glo
                        nc.tensor.matmul(
                            pms[i][0:m, :, :],
                            wg[:, c, :, o:o + m],
                            fts[c][:, :, :, pfirst:pfirst + win],
                            start=(c == 0),
                            stop=(c == NC2 - 1),
                            perf_mode=DR,
                        )
                # fused mask-mult + window-reduce on DVE (the only engine
                # that can do tensor*tensor reads from PSUM; offloading the
                # reduce to Act still leaves DVE paying the PSUM-read
                # multiply, so a split buys nothing)
                for i in blks:
                    m, win = ms[i], wins[i]
                    mo = int(moffs[i])
                    for bb in range(BPC):
                        sc = spool.tile([128, WINMAX], F32, tag=f"sv{bb}")
                        nc.vector.scalar_tensor_tensor(
                            sc[0:m, 0:win],
                            pms[i][0:m, bb, :],
                            0.0,
                            mask_t[0:m, mo:mo + win],
                            ADD,
                            MULT,
                            accum_out=zAll[0:m, BPC * i + bb:BPC * i + bb + 1],
                        )
                # store this group's partials right away (overlaps the
                # rest); the final group stores per block so the very last
                # DMA waits only on the last block's reduce
                if g == ngrp - 1:
                    for i in blks:
                        sl = slice(BPC * i, BPC * (i + 1))
                        seng = nc.sync if (i % 2 == 0) else nc.scalar
                        seng.dma_start(z_h[:, sl], zAll[:, sl])
                else:
                    sl = slice(BPC * gbounds[g], BPC * gbounds[g + 1])
                    seng = nc.sync if (g % 2 == 0) else nc.scalar
                    seng.dma_start(z_h[:, sl], zAll[:, sl])

    _split_waits(nc)

    # ---- run on 8 cores: core id = bg*2 + dh ----
    xq = (x.reshape(B, D // 128, 128, P) * sx).astype(F8_NP)
    in_maps = []
    for core in range(NCORES):
        bg, dh = core // NDH, core % NDH
        xs_dev = np.zeros((128, NC2, 2, BPC, FEATW), F8_NP)
        # xq[b, chunk128, p, pos] -> [p, cc2, i, bb, pos]
        blkx = xq[BPC * bg:BPC * (bg + 1),
                  4 * dh:4 * (dh + 1)].reshape(BPC, NC2, 2, 128, P)
        xs_dev[:, :, :, :, :P] = blkx.transpose(3, 1, 2, 0, 4)
        in_maps.append({
            "xs": xs_dev,
            "wf": walls[dh],
            "mf": mask_all,
        })
    res = run_bass_kernel_spmd(nc, in_maps, core_ids=list(range(NCORES)))

    # ---- assemble: add D-halves, bias, elu(y)+1 ----
    y = np.empty((B, N), np.float32)
    for bg in range(NBG):
        z = res.results[NDH * bg]["z"] + res.results[NDH * bg + 1]["z"]
        for i, (s, e) in enumerate(blocks):
            idx = order[s:e]
            m = e - s
            y[BPC * bg:BPC * (bg + 1), idx] = z[0:m, BPC * i:BPC * (i + 1)].T
    y += b
    return np.where(y > 0, y + np.float32(1.0),
                    np.exp(np.minimum(y, np.float32(0.0)))).astype(np.float32)


# revision 38
# speedup vs baseline: 1.1504x; 1.0187x over previous
"""PoissonGaussianReadout forward on 8 trn2 NeuronCores.

Math (eval mode): each neuron n samples feat[b] (a [36,36,1024] image per
batch, 1024 = C*T channels) bilinearly at a fixed point mu[n], then takes a
per-neuron dot with W[n,:], adds b[n], applies elu(y)+1.

Strategy:
  - Hybrid shard 4x2: 8 cores = 4 batch-groups (4 batches each) x 2 halves
    of the contraction dim D (512 channels each).  Cores emit LINEAR
    partial sums; the host adds the halves, bias, and elu on [16,4096].
    (Splitting D halves the per-core W traffic; splitting batches keeps
    the per-core feat traffic at 1/8 -- together they minimize both the
    DMA stream and the feat-arrival gate.)
  - fp8(e4m3) x and W with DoubleRow matmuls: x uses one global scale, W a
    per-neuron scale; both dequant factors fold into the (per-neuron) mask.
    Halves both the DMA stream and the PE time vs bf16 (rel err 1.3e-2,
    within the 2e-2 gate; inputs are deterministic).
  - Sort neurons by bilinear base cell p00 = y0*36+x0; blocks of <=128
    sorted neurons span a window of <=WINMAX flat positions.  Two
    DoubleRow matmuls per block (256-channel subtile pairs):
    psum[n, (b,j)] += Wblk^T @ feat-window, a contiguous slice of feat.
  - Each neuron's 4 bilinear corners live at window offsets
    (p00-pfirst)+{0,1,36,37}; a host-built sparse mask [n, win] (bf16,
    fp8 dequant folded in) holds the bilinear weights.  One DVE
    scalar_tensor_tensor per (block, batch) fuses mask-multiply and
    window-reduce straight out of PSUM into z.  This DVE phase (~21us)
    is the critical path: it is the only engine that can do
    tensor*tensor reads from PSUM, and its cost is bound by
    per-instruction window elements + accumulator drains.
  - DMA is need-ordered on the two HWDGE queues: one contiguous feat
    double-chunk per queue first (the whole PE/DVE pipeline gates on
    feat), then W block-groups + their mask slices just-in-time behind
    PE consumption; per-group z stores overlap the pipeline.
"""
import sys
sys.path.insert(0, "/opt/trn_rl_repo")

import numpy as np

from concourse import bass, mybir, tile
from concourse.bass_utils import run_bass_kernel_spmd
import bass_rust

# problem constants
B, C, T, HH, WW = 16, 64, 16, 36, 36
N, D = 4096, C * T             # 4096 neurons, 1024 input dim
P = HH * WW                    # 1296 flat positions
NCORES = 8
NBG = 4                        # batch groups
NDH = 2                        # D halves
BPC = B // NBG                 # batches per core = 4
DH = D // NDH                  # channels per core = 512
NC2 = DH // 256                # 2 double-subtile (256-chan) passes per core
PAD = 38                       # max corner offset (37) + 1
WINMAX = 128                   # psum bank: BPC*WIN <= 512 fp32
FEATW = P + PAD                # padded feat width per (chunk, batch)
GRPN = 4                       # blocks per W DMA group

F32 = mybir.dt.float32

import ml_dtypes
F8_DT = mybir.dt.float8e4
F8_NP = ml_dtypes.float8_e4m3   # max normal 240
F8_CAP = np.float32(224.0)


def _split_waits(nc, max_waits=1):
    """Walrus in this image allows only ONE sem wait per instruction.
    Hoist extra waits onto injected same-engine NoOps placed immediately
    before the owning instruction (same engine + program order => same
    semantics)."""
    k = 0
    for fn in nc.m.functions:
        for blk in fn.blocks:
            insts = blk.instructions
            out = []
            for inst in insts:
                si = inst.sync_info
                if si is not None and si.on_wait and len(si.on_wait) > max_waits:
                    waits = list(si.on_wait)
                    for w in waits[:-max_waits]:
                        nop = mybir.InstNoOp(name=f"I-wsplit-{k}", ins=[], outs=[])
                        k += 1
                        nop.engine = inst.engine
                        nop.sync_info = bass_rust.SyncInfo(
                            on_wait=[w], on_update=[]
                        )
                        out.append(nop)
                    si.on_wait = waits[-max_waits:]
                    inst.sync_info = si
                out.append(inst)
            if len(out) != len(insts):
                insts.clear()
                insts.extend(out)


def _bilinear_tables(mu):
    """Per-neuron base cell p00, corner offsets (4) in {0,1,36,37}, corner
    weights (4), replicating reference float32 arithmetic exactly."""
    one, half = np.float32(1.0), np.float32(0.5)
    g = np.clip(mu.astype(np.float32), -one, one)
    ix = (g[:, 0] + one) * np.float32(WW * 0.5) - half
    iy = (g[:, 1] + one) * np.float32(HH * 0.5) - half
    x0 = np.floor(ix)
    y0 = np.floor(iy)
    wx1 = ix - x0
    wx0 = one - wx1
    wy1 = iy - y0
    wy0 = one - wy1

    xs = [x0, x0 + one]
    ys = [y0, y0 + one]
    wxs = [wx0, wx1]
    wys = [wy0, wy1]

    x0c = np.clip(x0, 0, WW - 1).astype(np.int64)
    y0c = np.clip(y0, 0, HH - 1).astype(np.int64)
    p00 = y0c * WW + x0c

    offs = np.zeros((4, N), np.int64)
    wgts = np.zeros((4, N), np.float32)
    k = 0
    for a in range(2):          # y corner
        for bb in range(2):     # x corner
            xx, yy = xs[bb], ys[a]
            valid = (xx >= 0) & (xx <= WW - 1) & (yy >= 0) & (yy <= HH - 1)
            xi = np.clip(xx, 0, WW - 1).astype(np.int64)
            yi = np.clip(yy, 0, HH - 1).astype(np.int64)
            offs[k] = yi * WW + xi - p00
            wgts[k] = (wys[a] * wxs[bb]) * valid.astype(np.float32)
            k += 1
    assert offs.min() >= 0 and offs.max() <= 37
    return p00, offs, wgts


def _make_blocks(p00_sorted):
    """Greedy blocks of <=128 sorted neurons with window <= WINMAX."""
    blocks = []  # (start, end) into sorted order
    s = 0
    n = len(p00_sorted)
    while s < n:
        pfirst = p00_sorted[s]
        e = s
        while e < n and e - s < 128 and (p00_sorted[e] - pfirst) + PAD <= WINMAX:
            e += 1
        blocks.append((s, e))
        s = e
    return blocks


def kernel(x, mu, sigma, W, b):
    x = np.ascontiguousarray(x, dtype=np.float32)
    W = np.ascontiguousarray(W, dtype=np.float32)
    b = np.asarray(b, dtype=np.float32)

    p00, offs, wgts = _bilinear_tables(mu)
    order = np.argsort(p00, kind="stable")
    p00s = p00[order]
    blocks = _make_blocks(p00s)
    nblk = len(blocks)

    # ---- fp8 quantization: global x scale, per-neuron W scale ----
    sx = F8_CAP / np.float32(max(np.abs(x).max(), 1e-30))
    sw = F8_CAP / np.maximum(np.abs(W).max(axis=1), 1e-30).astype(np.float32)
    Wq = (W * sw[:, None]).astype(F8_NP)    # [N, D]
    dequant = 1.0 / (sw * sx)               # [N] folded into the mask

    # per-block host data
    wins, pfirsts, ms, sblk = [], [], [], []
    mparts = []
    for i, (s, e) in enumerate(blocks):
        idx = order[s:e]
        m = e - s
        pfirst = int(p00s[s])
        win = int(p00s[e - 1]) - pfirst + PAD
        ms.append(m)
        pfirsts.append(pfirst)
        wins.append(win)
        sblk.append(s)
        # mask [128, win], fp8 dequant folded in
        mk = np.zeros((128, win), np.float32)
        rel = (p00[idx] - pfirst)  # [m]
        for k in range(4):
            np.add.at(mk[:m], (np.arange(m), rel + offs[k][idx]),
                      wgts[k][idx] * dequant[idx])
        mparts.append(mk)
    sblk.append(N)

    # W groups: first small so the PE can start early, the rest sized GRPN
    gbounds = [0, min(2, nblk)]
    while gbounds[-1] + GRPN < nblk:
        gbounds.append(gbounds[-1] + GRPN)
    if gbounds[-1] < nblk:
        gbounds.append(nblk)
    ngrp = len(gbounds) - 1

    # W packed per group with one contiguous row per partition:
    # group layout [128, NC2, 2, sum_m(group)]; groups concatenated flat.
    Ws = Wq[order]                          # [N, D] sorted
    gw_off = [0]
    for g in range(ngrp):
        gm = sblk[gbounds[g + 1]] - sblk[gbounds[g]]
        gw_off.append(gw_off[-1] + NC2 * 2 * gm)
    walls = []
    for dh in range(NDH):
        wl = (Ws[:, dh * DH:(dh + 1) * DH].T        # [512, N]
              .reshape(NC2, 2, 128, N).transpose(2, 0, 1, 3))  # [128,NC2,2,N]
        parts = []
        for g in range(ngrp):
            lo, hi = sblk[gbounds[g]], sblk[gbounds[g + 1]]
            parts.append(wl[:, :, :, lo:hi].reshape(128, -1))
        walls.append(np.ascontiguousarray(np.concatenate(parts, axis=1)))
    mask_all = np.ascontiguousarray(
        np.concatenate(mparts, axis=1)).astype(ml_dtypes.bfloat16)
    moffs = np.cumsum([0] + [w for w in wins])
    mtot = int(mask_all.shape[1])

    # ---- build the Bass program (same for all cores) ----
    nc = bass.Bass()
    xs_h = nc.declare_dram_parameter("xs", [128, NC2, 2, BPC, FEATW], F8_DT,
                                     isOutput=False)
    wf_h = nc.declare_dram_parameter("wf", [128, int(gw_off[-1])], F8_DT,
                                     isOutput=False)
    mf_h = nc.declare_dram_parameter("mf", [128, mtot], mybir.dt.bfloat16,
                                    isOutput=False)
    z_h = nc.declare_dram_parameter("z", [128, BPC * nblk], F32, isOutput=True)

    ADD = mybir.AluOpType.add
    MULT = mybir.AluOpType.mult
    DR = mybir.MatmulPerfMode.DoubleRow

    with tile.TileContext(nc) as tc:
        with (
            tc.tile_pool(name="feat", bufs=1) as featp,
            tc.tile_pool(name="wpool", bufs=1) as wpool,
            tc.tile_pool(name="mpool", bufs=1) as mpool,
            tc.tile_pool(name="spool", bufs=4) as spool,
            tc.tile_pool(name="zpool", bufs=1) as zpool,
            tc.tile_pool(name="psum", bufs=1, space="PSUM") as psump,
        ):
            fts = [featp.tile([128, 2, BPC, FEATW], F8_DT, name=f"feat{c}")
                   for c in range(NC2)]
            mask_t = mpool.tile([128, mtot], mybir.dt.bfloat16)
            zAll = zpool.tile([128, BPC * nblk], F32)
            wgs = {}
            for g in range(ngrp):
                gcols = sblk[gbounds[g + 1]] - sblk[gbounds[g]]
                wgs[g] = wpool.tile([128, NC2, 2, gcols], F8_DT, name=f"wg{g}")

            def wg_item(g):
                return (wgs[g][:], wf_h[:, int(gw_off[g]):int(gw_off[g + 1])])

            def mask_item(g):
                lo = int(moffs[gbounds[g]])
                hi = int(moffs[gbounds[g + 1]])
                return (mask_t[:, lo:hi], mf_h[:, lo:hi])

            # One contiguous DMA per feat double-chunk (10.7KB rows, full
            # DMA rate), one per queue in parallel; then W0+mask0 split
            # across both queues (earliest PE start), then the remaining
            # groups + mask slices alternating queues need-ordered.
            # One contiguous DMA per feat double-chunk (10.7KB rows, full
            # DMA rate), one per queue in parallel.  W groups follow as
            # single whole-group DMAs (small-row splits cost ~1us each in
            # per-descriptor overhead and delayed the PE start by 2.3us).
            # Masks merge into just two DMAs placed by first need time.
            mhalf = int(moffs[gbounds[min(2, ngrp)]])
            sync_items = [[(fts[0][:], xs_h[:, 0])]]
            scal_items = [[(fts[1][:], xs_h[:, 1])],
                          [(mask_t[:, 0:mhalf], mf_h[:, 0:mhalf])]]
            for g in range(ngrp):
                (sync_items if (g % 2 == 0) else scal_items).append(
                    [wg_item(g)]
                )
            scal_items.insert(3, [(mask_t[:, mhalf:mtot],
                                   mf_h[:, mhalf:mtot])])
            plan = {nc.sync: sync_items, nc.scalar: scal_items}
            maxlen = max(len(v) for v in plan.values())
            for k in range(maxlen):
                for eng, items in plan.items():
                    if k < len(items):
                        for dst, srcap in items[k]:
                            eng.dma_start(dst, srcap)

            for g in range(ngrp):
                blks = list(range(gbounds[g], gbounds[g + 1]))
                wg = wgs[g]
                glo = sblk[gbounds[g]]
                pms = {}
                for i in blks:
                    pms[i] = psump.tile([128, BPC, wins[i]], F32,
                                        name=f"pm{i}", tag=f"pm{i % 8}")
                for c in range(NC2):
                    for i in blks:
                        m, win, pfirst = ms[i], wins[i], pfirsts[i]
                        o = sblk[i] - glo
                        nc.tensor.matmul(
                            pms[i][0:m, :, :],
                            wg[:, c, :, o:o + m],
                            fts[c][:, :, :, pfirst:pfirst + win],
                            start=(c == 0),
                            stop=(c == NC2 - 1),
                            perf_mode=DR,
                        )
                # fused mask-mult + window-reduce on DVE (the only engine
                # that can do tensor*tensor reads from PSUM; offloading the
                # reduce to Act still leaves DVE paying the PSUM-read
                # multiply, so a split buys nothing)
                for i in blks:
                    m, win = ms[i], wins[i]
                    mo = int(moffs[i])
                    for bb in range(BPC):
                        sc = spool.tile([128, WINMAX], F32, tag=f"sv{bb}")
                        nc.vector.scalar_tensor_tensor(
                            sc[0:m, 0:win],
                            pms[i][0:m, bb, :],
                            0.0,
                            mask_t[0:m, mo:mo + win],
                            ADD,
                            MULT,
                            accum_out=zAll[0:m, BPC * i + bb:BPC * i + bb + 1],
                        )
                # store this group's partials right away (overlaps the
                # rest); the final group stores per block so the very last
                # DMA waits only on the last block's reduce
                if g == ngrp - 1:
                    for i in blks:
                        sl = slice(BPC * i, BPC * (i + 1))
                        seng = nc.sync if (i % 2 == 0) else nc.scalar
                        seng.dma_start(z_h[:, sl], zAll[:, sl])
                else:
                    sl = slice(BPC * gbounds[g], BPC * gbounds[g + 1])
                    seng = nc.sync if (g % 2 == 0) else nc.scalar
                    seng.dma_start(z_h[:, sl], zAll[:, sl])

    _split_waits(nc)

    # ---- run on 8 cores: core id = bg*2 + dh ----
    xq = (x.reshape(B, D // 128, 128, P) * sx).astype(F8_NP)
    in_maps = []
    for core in range(NCORES):
        bg, dh = core // NDH, core % NDH
        xs_dev = np.zeros((128, NC2, 2, BPC, FEATW), F8_NP)
        # xq[b, chunk128, p, pos] -> [p, cc2, i, bb, pos]
        blkx = xq[BPC * bg:BPC * (bg + 1),
                  4 * dh:4 * (dh + 1)].reshape(BPC, NC2, 2, 128, P)
        xs_dev[:, :, :, :, :P] = blkx.transpose(3, 1, 2, 0, 4)
        in_maps.append({
            "xs": xs_dev,
            "wf": walls[dh],
            "mf": mask_all,
        })
    res = run_bass_kernel_spmd(nc, in_maps, core_ids=list(range(NCORES)))

    # ---- assemble: add D-halves, bias, elu(y)+1 ----
    y = np.empty((B, N), np.float32)
    for bg in range(NBG):
        z = res.results[NDH * bg]["z"] + res.results[NDH * bg + 1]["z"]
        for i, (s, e) in enumerate(blocks):
            idx = order[s:e]
            m = e - s
            y[BPC * bg:BPC * (bg + 1), idx] = z[0:m, BPC * i:BPC * (i + 1)].T
    y += b
    return np.where(y > 0, y + np.float32(1.0),
                    np.exp(np.minimum(y, np.float32(0.0)))).astype(np.float32)
